# revision 45
# baseline (speedup 1.0000x reference)
"""v4: spatial-quarters layout.

Partitions = 4 spatial quarters x 32 channels (always 128, fully packed).
k/v live as per-quarter haloed padded grids kg/vg [128, 22, 70]; tap (i, j)
is the pure strided slice [:, i:i+16, j:j+64].  q is quartered to [128, 1024].

Per (i, j-group) iteration (j-groups: j=0..3 and j=4..6):
- one batched DVE tensor_scalar add (kb = kg_taps + rel_i) over an
  overlapping custom AP covering all nj j-offsets at once,
- one batched DVE multiply pb = kb * q (q broadcast over j via stride-0 AP),
  (or one fused Pool scalar_tensor_tensor for offloaded groups),
- one ACT exp over the whole group,
- one batched DVE/Pool multiply ev = eb * vg_taps,
- per tap: 4 identity matmuls (2x512 cols for each of SE/SEV) accumulating
  straight into PSUM over all 49 taps.

Tail: out = SEV * reciprocal(SE) on the same partitions (no DMA hop), 4
quarter DMAs out.  Projections run on PE in float32r (1 cycle/row).

Sharding: 8 cores = batch(4) x channel-half(2); the half=1 core sees
H/W-transposed inputs so its rel_w bias becomes a row (i) bias too.
"""
import sys

sys.path.insert(0, "/opt/trn_rl_repo")

import numpy as np
import ml_dtypes

import bass_rust
import concourse.bass as bass
import concourse.bacc as bacc
import concourse.tile as tile
from concourse import mybir
from concourse.bass_utils import run_bass_kernel_spmd

F32 = mybir.dt.float32
F32R = mybir.dt.float32r
BF16 = mybir.dt.bfloat16
AF = mybir.ActivationFunctionType
ALU = mybir.AluOpType

B, C, H, W = 4, 64, 64, 64
CO = 32
K = 7
PG = 70            # padded grid side
SP = H * W         # 4096
NQ = 4             # spatial quarters
QR = H // NQ       # 16 rows per quarter
QS = QR * W        # 1024 spatial per quarter
GR = QR + K - 1    # 22 haloed grid rows per quarter

# (i, g) groups whose logit / E*v work runs on Pool instead of DVE
# (Pool has no TensorScalarPtr/STT support on TRN2 ISA — only plain TT)
POOL_LOGIT = set()
# last group's EV stays on DVE: a slow Pool EV at the end stretches the drain
POOL_EV = {(i, 1) for i in range(K - 3)}


def _sp_view(t, sl):
    """[128, 16, 64] view of a quarter-spatial slice of a flat tile."""
    return t[:, sl].rearrange("p (r w) -> p r w", w=W)


def build_graph():
    nc = bacc.Bacc(None, target_bir_lowering=False)
    xs_d = nc.dram_tensor("xs", [C, SP], F32, kind="ExternalInput")
    ys_d = nc.dram_tensor("ys", [C, SP], F32, kind="ExternalInput")
    # wall = [wk | wq | wv] fused; rlq pre-replicated to 128 partitions
    wall_d = nc.dram_tensor("wall", [C, 3 * CO], F32, kind="ExternalInput")
    rlq_d = nc.dram_tensor("rlq", [128, K], F32, kind="ExternalInput")
    id_d = nc.dram_tensor("id128", [128, 128], BF16, kind="ExternalInput")
    out_d = nc.dram_tensor("out", [CO, SP], F32, kind="ExternalOutput")

    with tile.TileContext(nc) as tc:
        with (
            tc.tile_pool(name="sb", bufs=1) as sb,
            tc.tile_pool(name="ps", bufs=1, space="PSUM") as psp,
        ):
            wall = sb.tile([C, 3 * CO], F32R)
            rlq = sb.tile([128, K], F32)
            idm = sb.tile([128, 128], BF16)
            kg = sb.tile([128, GR, PG], BF16)
            vg = sb.tile([128, GR, PG], BF16)
            qq = sb.tile([128, QS], BF16)
            stg = sb.tile([CO, PG, PG], BF16)  # shared k/v evac staging
            # q staging lives on partitions 32:64 (evac from pj[32:64] cannot
            # cross partitions); only rows 32:64 are used
            qstg = sb.tile([2 * CO, SP], BF16)
            kb = sb.tile([128, NQ * QS], BF16)
            NBUF = 5
            PB = [sb.tile([128, NQ * QS], BF16, name=f"PB{t}", tag=f"PB{t}") for t in range(NBUF)]
            EB = [sb.tile([128, NQ * QS], BF16, name=f"EB{t}", tag=f"EB{t}") for t in range(NBUF)]
            EVt = [sb.tile([128, NQ * QS], BF16, name=f"EV{t}", tag=f"EV{t}") for t in range(NBUF)]
            rcp = sb.tile([128, QS], F32)
            outf = sb.tile([128, QS], F32)
            SE = psp.tile([128, QS], F32)
            SEV = psp.tile([128, QS], F32)
            PRJ = [psp.tile([128, 512], F32, name=f"PRJ{t}", tag=f"PRJ{t}") for t in range(4)]

            xs_sb = sb.tile([C, SP], F32R)
            ys_sb = sb.tile([C, SP], F32R)
            nc.sync.dma_start(out=wall[:, :], in_=wall_d[:, :].bitcast(F32R))
            nc.sync.dma_start(out=xs_sb[:, 0:512], in_=xs_d[:, 0:512].bitcast(F32R))
            nc.sync.dma_start(out=xs_sb[:, 512:2048], in_=xs_d[:, 512:2048].bitcast(F32R))
            nc.sync.dma_start(out=xs_sb[:, 2048:SP], in_=xs_d[:, 2048:SP].bitcast(F32R))
            nc.gpsimd.dma_start(out=rlq[:, :], in_=rlq_d[:, :])
            nc.gpsimd.dma_start(out=idm[:, :], in_=id_d[:, :])
            nc.gpsimd.dma_start(out=ys_sb[:, :], in_=ys_d[:, :].bitcast(F32R))

            # pad borders of the (shared) staging grid once; k/q/v evacs only
            # ever write the interior
            nc.gpsimd.memset(stg[:, 0:3, :], 0.0)
            nc.gpsimd.memset(stg[:, 3 + H:PG, :], 0.0)
            nc.gpsimd.memset(stg[:, 3:3 + H, 0:3], 0.0)
            nc.gpsimd.memset(stg[:, 3:3 + H, 3 + W:PG], 0.0)

            # combined k+q projection: stationary [wk|wq] -> 64 out partitions
            # (f32r dst still at PSUM partition 0); k evac on ACT, q on DVE
            def kg_bcast(s, eng):
                eng.dma_start(out=kg[32 * s:32 * s + 32, :, :],
                              in_=stg[:, QR * s:QR * s + GR, :])

            def qq_bcast(s, eng):
                eng.dma_start(out=qq[32 * s:32 * s + 32, :],
                              in_=qstg[CO:2 * CO, s * QS:(s + 1) * QS])

            # per-quarter broadcasts fire as soon as their evac chunks land
            # (quarter s of kg needs chunks <= {2,4,6,7}; of qq chunks 2s,2s+1)
            for cch in range(8):
                cs = slice(cch * 512, (cch + 1) * 512)
                pj = PRJ[cch % 4]
                nc.tensor.matmul(pj[0:2 * CO, :], wall[:, 0:2 * CO],
                                 xs_sb[:, cs], start=True, stop=True)
                nc.scalar.copy(
                    out=stg[:, 3 + 8 * cch:3 + 8 * cch + 8, 3:3 + W],
                    in_=pj[0:CO, :].rearrange("p (r w) -> p r w", w=W))
                nc.vector.tensor_copy(qstg[CO:2 * CO, cs], pj[CO:2 * CO, :])
                if cch == 1:
                    qq_bcast(0, nc.sync)
                elif cch == 2:
                    kg_bcast(0, nc.gpsimd)
                elif cch == 3:
                    qq_bcast(1, nc.sync)
                elif cch == 4:
                    kg_bcast(1, nc.gpsimd)
                elif cch == 5:
                    qq_bcast(2, nc.sync)
                elif cch == 6:
                    kg_bcast(2, nc.gpsimd)
            qq_bcast(3, nc.sync)
            kg_bcast(3, nc.gpsimd)
            # v projection reuses stg after the k broadcast; emitted lazily
            # (after the first two groups' logits) so its DVE evacs don't
            # delay the main loop's start in the in-order DVE queue
            def emit_v_proj():
                for cch in range(8):
                    cs = slice(cch * 512, (cch + 1) * 512)
                    pj = PRJ[cch % 4]
                    nc.tensor.matmul(pj[0:CO, :], wall[:, 2 * CO:3 * CO],
                                     ys_sb[:, cs], start=True, stop=True)
                    nc.vector.tensor_copy(
                        stg[:, 3 + 8 * cch:3 + 8 * cch + 8, 3:3 + W],
                        pj[0:CO, :].rearrange("p (r w) -> p r w", w=W))
                for s in range(NQ):
                    eng = nc.sync if s % 2 == 0 else nc.gpsimd
                    eng.dma_start(out=vg[32 * s:32 * s + 32, :, :],
                                  in_=stg[:, QR * s:QR * s + GR, :])

            # ---------------- main loop over row-taps i and j-groups
            # EV + matmuls are emitted with a 2-group lag so the in-order DVE
            # stream never parks behind the exp each EV depends on.
            qqv = _sp_view(qq, slice(0, QS))
            groups = [(i, g, j0, nj) for i in range(K)
                      for g, (j0, nj) in enumerate(((0, 4), (4, 3)))]

            def emit_ev_mm(idx):
                i, g, j0, nj = groups[idx]
                eb, ev = EB[idx % NBUF], EVt[idx % NBUF]
                for jj in range(nj):
                    j = j0 + jj
                    sl = slice(jj * QS, (jj + 1) * QS)
                    eng = nc.gpsimd if (i, g) in POOL_EV else nc.vector
                    eng.tensor_tensor(
                        _sp_view(ev, sl), _sp_view(eb, sl),
                        vg[:, i:i + QR, j:j + W], ALU.mult)
                for acc, buf in ((SE, eb), (SEV, ev)):
                    for jj in range(nj):
                        st = (i == 0 and j0 + jj == 0)
                        sp = (i == K - 1 and j0 + jj == K - 1)
                        for hh in range(2):
                            cs = slice(jj * QS + hh * 512,
                                       jj * QS + hh * 512 + 512)
                            hs = slice(hh * 512, hh * 512 + 512)
                            nc.tensor.matmul(acc[:, hs], idm[:, :],
                                             buf[:, cs], start=st, stop=sp)

            for idx, (i, g, j0, nj) in enumerate(groups):
                pb, eb = PB[idx % NBUF], EB[idx % NBUF]
                # one biased-grid TSP covering all nj overlapping j-windows
                wj = W + nj - 1
                kbv = kb[:, 0:QR * wj].rearrange("p (r w) -> p r w", w=wj)
                nc.vector.tensor_scalar_add(
                    kbv, kg[:, i:i + QR, j0:j0 + wj], rlq[:, i:i + 1])
                for jj in range(nj):
                    sl = slice(jj * QS, (jj + 1) * QS)
                    eng = nc.gpsimd if (i, g) in POOL_LOGIT else nc.vector
                    eng.tensor_tensor(
                        _sp_view(pb, sl), kbv[:, :, jj:jj + W], qqv, ALU.mult)
                eh = (nj * QS) // 2
                nc.scalar.activation(eb[:, 0:eh], pb[:, 0:eh], AF.Exp)
                nc.scalar.activation(eb[:, eh:nj * QS], pb[:, eh:nj * QS], AF.Exp)
                if idx == 1:
                    emit_v_proj()
                if idx >= 2:
                    emit_ev_mm(idx - 2)
            emit_ev_mm(len(groups) - 2)
            emit_ev_mm(len(groups) - 1)

            # ---------------- division tail + output
            for hh in range(2):
                hs = slice(hh * 512, hh * 512 + 512)
                nc.vector.reciprocal(rcp[:, hs], SE[:, hs])
                nc.vector.tensor_tensor(outf[:, hs], SEV[:, hs], rcp[:, hs],
                                        ALU.mult)
            for s in range(NQ):
                eng = nc.sync if s % 2 == 0 else nc.gpsimd
                eng.dma_start(out=out_d[:, s * QS:(s + 1) * QS],
                              in_=outf[32 * s:32 * s + 32, :])
    nc.finalize()
    return nc


_nc_cache = None


def kernel(x, y, Wq, Wk, Wv, rel_h, rel_w, _trace=False):
    global _nc_cache
    if _nc_cache is None:
        _nc_cache = build_graph()
    nc = _nc_cache

    x = np.asarray(x, np.float32)
    y = np.asarray(y, np.float32)
    bf = ml_dtypes.bfloat16
    id128 = np.ascontiguousarray(np.eye(128, dtype=np.float32).astype(bf))
    in_maps = []
    for b in range(B):
        for half in range(2):
            sl = slice(half * CO, (half + 1) * CO)
            if half == 0:
                xs, ys = x[b], y[b]
                rel = np.asarray(rel_h, np.float32)
            else:
                xs = np.ascontiguousarray(x[b].transpose(0, 2, 1))
                ys = np.ascontiguousarray(y[b].transpose(0, 2, 1))
                rel = np.asarray(rel_w, np.float32)
            wall = np.concatenate(
                [np.asarray(Wk, np.float32)[sl].T,
                 np.asarray(Wq, np.float32)[sl].T,
                 np.asarray(Wv, np.float32)[sl].T], axis=1)
            in_maps.append({
                "xs": np.ascontiguousarray(xs.reshape(C, SP)),
                "ys": np.ascontiguousarray(ys.reshape(C, SP)),
                "wall": np.ascontiguousarray(wall),
                "rlq": np.ascontiguousarray(np.tile(rel, (NQ, 1))),
                "id128": id128,
            })

    res = run_bass_kernel_spmd(nc, in_maps, core_ids=list(range(8)), trace=_trace)

    out = np.empty((B, 2 * CO, H, W), np.float32)
    idx = 0
    for b in range(B):
        for half in range(2):
            o = res.results[idx]["out"].reshape(CO, H, W)
            if half == 1:
                o = o.transpose(0, 2, 1)
            out[b, half * CO:(half + 1) * CO] = o
            idx += 1
    if _trace:
        return out, res
    return out


# revision 46
# speedup vs baseline: 1.0189x; 1.0189x over previous
"""v4: spatial-quarters layout.

Partitions = 4 spatial quarters x 32 channels (always 128, fully packed).
k/v live as per-quarter haloed padded grids kg/vg [128, 22, 70]; tap (i, j)
is the pure strided slice [:, i:i+16, j:j+64].  q is quartered to [128, 1024].

Per (i, j-group) iteration (j-groups: j=0..3 and j=4..6):
- one batched DVE tensor_scalar add (kb = kg_taps + rel_i) over an
  overlapping custom AP covering all nj j-offsets at once,
- one batched DVE multiply pb = kb * q (q broadcast over j via stride-0 AP),
  (or one fused Pool scalar_tensor_tensor for offloaded groups),
- one ACT exp over the whole group,
- one batched DVE/Pool multiply ev = eb * vg_taps,
- per tap: 4 identity matmuls (2x512 cols for each of SE/SEV) accumulating
  straight into PSUM over all 49 taps.

Tail: out = SEV * reciprocal(SE) on the same partitions (no DMA hop), 4
quarter DMAs out.  Projections run on PE in float32r (1 cycle/row).

Sharding: 8 cores = batch(4) x channel-half(2); the half=1 core sees
H/W-transposed inputs so its rel_w bias becomes a row (i) bias too.
"""
import sys

sys.path.insert(0, "/opt/trn_rl_repo")

import numpy as np
import ml_dtypes

import bass_rust
import concourse.bass as bass
import concourse.bacc as bacc
import concourse.tile as tile
from concourse import mybir
from concourse.bass_utils import run_bass_kernel_spmd

F32 = mybir.dt.float32
F32R = mybir.dt.float32r
BF16 = mybir.dt.bfloat16
AF = mybir.ActivationFunctionType
ALU = mybir.AluOpType

B, C, H, W = 4, 64, 64, 64
CO = 32
K = 7
PG = 70            # padded grid side
SP = H * W         # 4096
NQ = 4             # spatial quarters
QR = H // NQ       # 16 rows per quarter
QS = QR * W        # 1024 spatial per quarter
GR = QR + K - 1    # 22 haloed grid rows per quarter

# (i, g) groups whose logit / E*v work runs on Pool instead of DVE
# (Pool has no TensorScalarPtr/STT support on TRN2 ISA — only plain TT)
POOL_LOGIT = set()
# last group's EV stays on DVE: a slow Pool EV at the end stretches the drain
POOL_EV = {(i, 1) for i in range(K - 2)}


def _sp_view(t, sl):
    """[128, 16, 64] view of a quarter-spatial slice of a flat tile."""
    return t[:, sl].rearrange("p (r w) -> p r w", w=W)


def build_graph():
    nc = bacc.Bacc(None, target_bir_lowering=False)
    xs_d = nc.dram_tensor("xs", [C, SP], F32, kind="ExternalInput")
    ys_d = nc.dram_tensor("ys", [C, SP], F32, kind="ExternalInput")
    # wall = [wk | wq | wv] fused; rlq pre-replicated to 128 partitions
    wall_d = nc.dram_tensor("wall", [C, 3 * CO], F32, kind="ExternalInput")
    rlq_d = nc.dram_tensor("rlq", [128, K], F32, kind="ExternalInput")
    id_d = nc.dram_tensor("id128", [128, 128], BF16, kind="ExternalInput")
    out_d = nc.dram_tensor("out", [CO, SP], F32, kind="ExternalOutput")

    with tile.TileContext(nc) as tc:
        with (
            tc.tile_pool(name="sb", bufs=1) as sb,
            tc.tile_pool(name="ps", bufs=1, space="PSUM") as psp,
        ):
            wall = sb.tile([C, 3 * CO], F32R)
            rlq = sb.tile([128, K], F32)
            idm = sb.tile([128, 128], BF16)
            kg = sb.tile([128, GR, PG], BF16)
            vg = sb.tile([128, GR, PG], BF16)
            qq = sb.tile([128, QS], BF16)
            stg = sb.tile([CO, PG, PG], BF16)  # shared k/v evac staging
            # q staging lives on partitions 32:64 (evac from pj[32:64] cannot
            # cross partitions); only rows 32:64 are used
            qstg = sb.tile([2 * CO, SP], BF16)
            kb = sb.tile([128, NQ * QS], BF16)
            NBUF = 5
            PB = [sb.tile([128, NQ * QS], BF16, name=f"PB{t}", tag=f"PB{t}") for t in range(NBUF)]
            EB = [sb.tile([128, NQ * QS], BF16, name=f"EB{t}", tag=f"EB{t}") for t in range(NBUF)]
            EVt = [sb.tile([128, NQ * QS], BF16, name=f"EV{t}", tag=f"EV{t}") for t in range(NBUF)]
            rcp = sb.tile([128, QS], F32)
            outf = sb.tile([128, QS], F32)
            SE = psp.tile([128, QS], F32)
            SEV = psp.tile([128, QS], F32)
            PRJ = [psp.tile([128, 512], F32, name=f"PRJ{t}", tag=f"PRJ{t}") for t in range(4)]

            xs_sb = sb.tile([C, SP], F32R)
            ys_sb = sb.tile([C, SP], F32R)
            nc.sync.dma_start(out=wall[:, :], in_=wall_d[:, :].bitcast(F32R))
            nc.sync.dma_start(out=xs_sb[:, 0:512], in_=xs_d[:, 0:512].bitcast(F32R))
            nc.sync.dma_start(out=xs_sb[:, 512:2048], in_=xs_d[:, 512:2048].bitcast(F32R))
            nc.sync.dma_start(out=xs_sb[:, 2048:SP], in_=xs_d[:, 2048:SP].bitcast(F32R))
            nc.gpsimd.dma_start(out=rlq[:, :], in_=rlq_d[:, :])
            nc.gpsimd.dma_start(out=idm[:, :], in_=id_d[:, :])
            nc.gpsimd.dma_start(out=ys_sb[:, :], in_=ys_d[:, :].bitcast(F32R))

            # pad borders of the (shared) staging grid once; k/q/v evacs only
            # ever write the interior
            nc.gpsimd.memset(stg[:, 0:3, :], 0.0)
            nc.gpsimd.memset(stg[:, 3 + H:PG, :], 0.0)
            nc.gpsimd.memset(stg[:, 3:3 + H, 0:3], 0.0)
            nc.gpsimd.memset(stg[:, 3:3 + H, 3 + W:PG], 0.0)

            # combined k+q projection: stationary [wk|wq] -> 64 out partitions
            # (f32r dst still at PSUM partition 0); k evac on ACT, q on DVE
            def kg_bcast(s, eng):
                eng.dma_start(out=kg[32 * s:32 * s + 32, :, :],
                              in_=stg[:, QR * s:QR * s + GR, :])

            def qq_bcast(s, eng):
                eng.dma_start(out=qq[32 * s:32 * s + 32, :],
                              in_=qstg[CO:2 * CO, s * QS:(s + 1) * QS])

            # per-quarter broadcasts fire as soon as their evac chunks land
            # (quarter s of kg needs chunks <= {2,4,6,7}; of qq chunks 2s,2s+1)
            for cch in range(8):
                cs = slice(cch * 512, (cch + 1) * 512)
                pj = PRJ[cch % 4]
                nc.tensor.matmul(pj[0:2 * CO, :], wall[:, 0:2 * CO],
                                 xs_sb[:, cs], start=True, stop=True)
                nc.scalar.copy(
                    out=stg[:, 3 + 8 * cch:3 + 8 * cch + 8, 3:3 + W],
                    in_=pj[0:CO, :].rearrange("p (r w) -> p r w", w=W))
                nc.vector.tensor_copy(qstg[CO:2 * CO, cs], pj[CO:2 * CO, :])
                if cch == 1:
                    qq_bcast(0, nc.sync)
                elif cch == 2:
                    kg_bcast(0, nc.gpsimd)
                elif cch == 3:
                    qq_bcast(1, nc.sync)
                elif cch == 4:
                    kg_bcast(1, nc.gpsimd)
                elif cch == 5:
                    qq_bcast(2, nc.sync)
                elif cch == 6:
                    kg_bcast(2, nc.gpsimd)
            qq_bcast(3, nc.sync)
            kg_bcast(3, nc.gpsimd)
            # v projection reuses stg after the k broadcast; emitted lazily
            # (after the first two groups' logits) so its DVE evacs don't
            # delay the main loop's start in the in-order DVE queue
            def emit_v_proj():
                for cch in range(8):
                    cs = slice(cch * 512, (cch + 1) * 512)
                    pj = PRJ[cch % 4]
                    nc.tensor.matmul(pj[0:CO, :], wall[:, 2 * CO:3 * CO],
                                     ys_sb[:, cs], start=True, stop=True)
                    nc.vector.tensor_copy(
                        stg[:, 3 + 8 * cch:3 + 8 * cch + 8, 3:3 + W],
                        pj[0:CO, :].rearrange("p (r w) -> p r w", w=W))
                for s in range(NQ):
                    eng = nc.sync if s % 2 == 0 else nc.gpsimd
                    eng.dma_start(out=vg[32 * s:32 * s + 32, :, :],
                                  in_=stg[:, QR * s:QR * s + GR, :])

            # ---------------- main loop over row-taps i and j-groups
            # EV + matmuls are emitted with a 2-group lag so the in-order DVE
            # stream never parks behind the exp each EV depends on.
            qqv = _sp_view(qq, slice(0, QS))
            groups = [(i, g, j0, nj) for i in range(K)
                      for g, (j0, nj) in enumerate(((0, 4), (4, 3)))]

            def emit_ev_mm(idx):
                i, g, j0, nj = groups[idx]
                eb, ev = EB[idx % NBUF], EVt[idx % NBUF]
                for jj in range(nj):
                    j = j0 + jj
                    sl = slice(jj * QS, (jj + 1) * QS)
                    eng = nc.gpsimd if (i, g) in POOL_EV else nc.vector
                    eng.tensor_tensor(
                        _sp_view(ev, sl), _sp_view(eb, sl),
                        vg[:, i:i + QR, j:j + W], ALU.mult)
                for acc, buf in ((SE, eb), (SEV, ev)):
                    for jj in range(nj):
                        st = (i == 0 and j0 + jj == 0)
                        sp = (i == K - 1 and j0 + jj == K - 1)
                        for hh in range(2):
                            cs = slice(jj * QS + hh * 512,
                                       jj * QS + hh * 512 + 512)
                            hs = slice(hh * 512, hh * 512 + 512)
                            nc.tensor.matmul(acc[:, hs], idm[:, :],
                                             buf[:, cs], start=st, stop=sp)

            for idx, (i, g, j0, nj) in enumerate(groups):
                pb, eb = PB[idx % NBUF], EB[idx % NBUF]
                # one biased-grid TSP covering all nj overlapping j-windows
                wj = W + nj - 1
                kbv = kb[:, 0:QR * wj].rearrange("p (r w) -> p r w", w=wj)
                nc.vector.tensor_scalar_add(
                    kbv, kg[:, i:i + QR, j0:j0 + wj], rlq[:, i:i + 1])
                for jj in range(nj):
                    sl = slice(jj * QS, (jj + 1) * QS)
                    eng = nc.gpsimd if (i, g) in POOL_LOGIT else nc.vector
                    eng.tensor_tensor(
                        _sp_view(pb, sl), kbv[:, :, jj:jj + W], qqv, ALU.mult)
                eh = (nj * QS) // 2
                nc.scalar.activation(eb[:, 0:eh], pb[:, 0:eh], AF.Exp)
                nc.scalar.activation(eb[:, eh:nj * QS], pb[:, eh:nj * QS], AF.Exp)
                if idx == 1:
                    emit_v_proj()
                if idx >= 2:
                    emit_ev_mm(idx - 2)
            emit_ev_mm(len(groups) - 2)
            emit_ev_mm(len(groups) - 1)

            # ---------------- division tail + output
            for hh in range(2):
                hs = slice(hh * 512, hh * 512 + 512)
                nc.vector.reciprocal(rcp[:, hs], SE[:, hs])
                nc.vector.tensor_tensor(outf[:, hs], SEV[:, hs], rcp[:, hs],
                                        ALU.mult)
            for s in range(NQ):
                eng = nc.sync if s % 2 == 0 else nc.gpsimd
                eng.dma_start(out=out_d[:, s * QS:(s + 1) * QS],
                              in_=outf[32 * s:32 * s + 32, :])
    nc.finalize()
    return nc


_nc_cache = None


def kernel(x, y, Wq, Wk, Wv, rel_h, rel_w, _trace=False):
    global _nc_cache
    if _nc_cache is None:
        _nc_cache = build_graph()
    nc = _nc_cache

    x = np.asarray(x, np.float32)
    y = np.asarray(y, np.float32)
    bf = ml_dtypes.bfloat16
    id128 = np.ascontiguousarray(np.eye(128, dtype=np.float32).astype(bf))
    in_maps = []
    for b in range(B):
        for half in range(2):
            sl = slice(half * CO, (half + 1) * CO)
            if half == 0:
                xs, ys = x[b], y[b]
                rel = np.asarray(rel_h, np.float32)
            else:
                xs = np.ascontiguousarray(x[b].transpose(0, 2, 1))
                ys = np.ascontiguousarray(y[b].transpose(0, 2, 1))
                rel = np.asarray(rel_w, np.float32)
            wall = np.concatenate(
                [np.asarray(Wk, np.float32)[sl].T,
                 np.asarray(Wq, np.float32)[sl].T,
                 np.asarray(Wv, np.float32)[sl].T], axis=1)
            in_maps.append({
                "xs": np.ascontiguousarray(xs.reshape(C, SP)),
                "ys": np.ascontiguousarray(ys.reshape(C, SP)),
                "wall": np.ascontiguousarray(wall),
                "rlq": np.ascontiguousarray(np.tile(rel, (NQ, 1))),
                "id128": id128,
            })

    res = run_bass_kernel_spmd(nc, in_maps, core_ids=list(range(8)), trace=_trace)

    out = np.empty((B, 2 * CO, H, W), np.float32)
    idx = 0
    for b in range(B):
        for half in range(2):
            o = res.results[idx]["out"].reshape(CO, H, W)
            if half == 1:
                o = o.transpose(0, 2, 1)
            out[b, half * CO:(half + 1) * CO] = o
            idx += 1
    if _trace:
        return out, res
    return out
